# revision 9
# baseline (speedup 1.0000x reference)
"""Trainium2 Bass kernel for nn_CausalLayer (bilinear causal mixing layer).

Math (per batch b):
    E = ae[x]                                # [L, D] gather
    S[i,j] = E_i @ w @ E_j                   # bilinear pairwise score
    coef[i,j] = (i+1)/(j+1) for i<j else 0
    res[:,j] = bx[:,j] + sum_i coef[i,j]*S[i,j]*bx[:,i]

Rather than materializing the [L, L] score matrix (O(L^2 H) flops), we use the
chunked linear-attention identity. With a_i = w^T E_i and y_i = (i+1)*bx_i:

    res_j = bx_j + (1/(j+1)) * [ M_cj @ E_j + sum_{i<j, same chunk} (a_i.E_j) y_i ]
    M_c   = sum_{i in chunks < c} y_i a_i^T      (rank-D running state, [D, H])

Per 128-token chunk that is: one masked [128,128] score block and three
[*,768] matmuls -- O(L*C*(D+H) + L*D*H) total, 16x fewer flops than the
reference einsum, which puts the kernel at the HBM roofline.

Key layout trick: the E/A rows are fetched with ONE transposed dma_gather per
batch (xbar transpose during the gather), so E^T and A^T arrive in SBUF in
matmul-operand layout [D, L] directly -- no per-chunk PE transposes or
PSUM->SBUF copies. bx loads ride the SP HWDGE ring, output stores the ACT
ring (the two rings drain in parallel). Output is written bf16 and upcast to
f32 on the host.

Sharding: batch-parallel, 2 of 16 batches per NeuronCore across 8 cores; ae/w
and the small constant tables are replicated. No cross-core communication.
"""

import os
import sys

for _p in ("/opt/trn_rl_repo", "/root/.axon_site/_ro/trn_rl_repo"):
    if os.path.isdir(_p) and _p not in sys.path:
        sys.path.insert(0, _p)

import numpy as np

B, L, H = 16, 2048, 768
V, D = 30000, 64
NCORES = 8
BPC = B // NCORES          # batches per core
C = 128                    # chunk (tile) size along sequence
NCH = L // C               # chunks per batch
ROWS = BPC * L             # bx rows per core
IOCH = 4                   # chunks per bx-load / out-store DMA
NI16 = L // 16             # idx columns per batch (16-partition wrap)

_compiled = {}


def _np_consts():
    import ml_dtypes

    i = np.arange(C, dtype=np.float64)
    cmask = np.zeros((C, NCH * C), np.float32)
    consts = np.zeros((C, 2 * NCH), np.float32)
    for c in range(NCH):
        gi = c * C + i
        cmask[:, c * C:(c + 1) * C] = np.where(
            i[:, None] < i[None, :], (gi + 1.0)[:, None], 0.0
        ).astype(np.float32)
        consts[:, c] = (gi + 1.0).astype(np.float32)
        consts[:, NCH + c] = (1.0 / (gi + 1.0)).astype(np.float32)
    return cmask.astype(ml_dtypes.bfloat16), consts


def _build():
    """Build + compile the per-core Bass module (SPMD: same program, 8 cores)."""
    key = "v2"
    if key in _compiled:
        return _compiled[key]

    import concourse.bacc as bacc
    import concourse.bass as bass
    import concourse.mybir as mybir
    import concourse.tile as tile
    from concourse.masks import make_identity

    f32 = mybir.dt.float32
    i16 = mybir.dt.int16
    bf16 = mybir.dt.bfloat16

    nc = bacc.Bacc(
        "TRN2",
        target_bir_lowering=False,
        debug=False,
        enable_asserts=False,
        num_devices=NCORES,
    )

    bx_d = nc.dram_tensor("bx", [ROWS, H], bf16, kind="ExternalInput").ap()
    # idx: [128, BPC*NI16] int16; batch b's tokens wrapped 16-partition style:
    # idx[p, b*NI16 + s] = x[b, s*16 + p%16]  (replicated 8x down partitions)
    idx_d = nc.dram_tensor("idx", [C, BPC * NI16], i16, kind="ExternalInput").ap()
    # fused gather table: row v = [ae[v] | (ae @ w)[v]] in bf16 (A = E @ w
    # precomputed on host; one transposed gather per batch delivers E^T/A^T)
    eaw_d = nc.dram_tensor("eaw", [V, 2 * D], bf16, kind="ExternalInput").ap()
    cm_d = nc.dram_tensor("cmask", [C, NCH * C], bf16, kind="ExternalInput").ap()
    ct_d = nc.dram_tensor("consts", [C, 2 * NCH], f32, kind="ExternalInput").ap()
    out_d = nc.dram_tensor("out", [ROWS, H], bf16, kind="ExternalOutput").ap()

    mult = mybir.AluOpType.mult
    add = mybir.AluOpType.add

    with tile.TileContext(nc) as tc:
        with (
            tc.tile_pool(name="const", bufs=1) as cpool,
            tc.tile_pool(name="bxp", bufs=3) as bxpool,
            tc.tile_pool(name="outp", bufs=2) as outpool,
            tc.tile_pool(name="sm", bufs=4) as smpool,
            tc.tile_pool(name="mp", bufs=2) as mpool,
            tc.tile_pool(name="ps_ap", bufs=2, space="PSUM") as ps_ap,
            tc.tile_pool(name="ps_s", bufs=2, space="PSUM") as ps_s,
            tc.tile_pool(name="ps_out", bufs=1, space="PSUM") as ps_out,
            tc.tile_pool(name="ps_m", bufs=1, space="PSUM") as ps_m,
        ):
            ident64 = cpool.tile([D, D], bf16)
            make_identity(nc, ident64[:])
            # idx first: the gathers wait on it
            idx_s = cpool.tile([C, BPC * NI16], i16)
            nc.sync.dma_start(out=idx_s[:], in_=idx_d[:, :])
            consts_s = cpool.tile([C, 2 * NCH], f32)
            nc.sync.dma_start(out=consts_s[:], in_=ct_d[:, :])

            # one transposed gather per batch:
            # EAt_b[q, 0, i] = eaw[x[b, i], q]  -> [2D, L] operand layout.
            # The A^T half lands on partitions 64-127; matmul needs lhsT/rhs at
            # the same base partition, so shift it down with one bulk DVE copy.
            # split into 512-index pieces: 32 descriptors per SDMA engine,
            # under the 64-desc packet ceiling
            GPIECE = 512
            EAt, At0 = [], []
            for b in range(BPC):
                ea = cpool.tile([C, L], bf16, name=f"EAt{b}")
                for q in range(L // GPIECE):
                    nc.gpsimd.dma_gather(
                        ea[:, q * GPIECE:(q + 1) * GPIECE].rearrange(
                            "p (one i) -> p one i", one=1
                        ),
                        eaw_d[:, :],
                        idx_s[
                            :,
                            b * NI16 + q * (GPIECE // 16):
                            b * NI16 + (q + 1) * (GPIECE // 16),
                        ],
                        GPIECE,
                        GPIECE,
                        2 * D,
                        transpose=True,
                    )
                EAt.append(ea)
                at0 = cpool.tile([D, L], bf16, name=f"At0{b}")
                nc.vector.tensor_scalar_mul(
                    out=at0[:], in0=ea[D:2 * D, :], scalar1=1.0
                )
                At0.append(at0)

            cmask_s = cpool.tile([C, NCH * C], bf16)
            nc.sync.dma_start(out=cmask_s[:, 0:8 * C], in_=cm_d[:, 0:8 * C])
            nc.sync.dma_start(out=cmask_s[:, 8 * C:], in_=cm_d[:, 8 * C:])

            for b in range(BPC):
                M_p = ps_m.tile([D, H], f32, name=f"M_p_b{b}", tag="M_p")
                for c in range(NCH):
                    g = b * NCH + c

                    # one DMA loads IOCH chunks' bx: [IOCH*128, H] -> [128, IOCH*H]
                    if c % IOCH == 0:
                        BXW = bxpool.tile([C, IOCH * H], bf16, name="BXW", tag="BXW")
                        nc.sync.dma_start(
                            out=BXW[:].rearrange("p (k h) -> p k h", k=IOCH),
                            in_=bx_d[g * C:(g + IOCH) * C, :].rearrange(
                                "(k p) h -> p k h", k=IOCH
                            ),
                        )
                    BX = BXW[:, (c % IOCH) * H:(c % IOCH + 1) * H]

                    if c > 0:
                        M_s = mpool.tile([D, H], bf16, name="M_s", tag="M_s")
                        nc.scalar.copy(out=M_s[:], in_=M_p[:])

                    Et = EAt[b][0:D, c * C:(c + 1) * C]
                    At = At0[b][:, c * C:(c + 1) * C]

                    # Ap = A * (i+1)  [C, D]: PE-transpose At, scale during the
                    # PSUM->SBUF move
                    ap_p = ps_ap.tile([C, D], bf16, name="ap_p", tag="ap_p")
                    nc.tensor.transpose(out=ap_p[:], in_=At, identity=ident64[:])
                    Ap = smpool.tile([C, D], bf16, name="Ap", tag="Ap")
                    nc.vector.tensor_scalar_mul(
                        out=Ap[:], in0=ap_p[:], scalar1=consts_s[:, c:c + 1]
                    )

                    # S = At^T @ Et  [C, C];  St = S * cmask_c
                    s_p = ps_s.tile([C, C], f32, name="s_p", tag="s_p")
                    nc.tensor.matmul(
                        out=s_p[:], lhsT=At, rhs=Et, start=True, stop=True,
                    )
                    St = smpool.tile([C, C], bf16, name="St", tag="St")
                    nc.vector.tensor_tensor(
                        out=St[:],
                        in0=s_p[:],
                        in1=cmask_s[:, c * C:(c + 1) * C],
                        op=mult,
                    )

                    # M += Ap^T @ BX  [D, H]  (skip the never-read last update).
                    # skip_group_check: the sim's group guard can't express this
                    # read-between-accumulations pattern; the pending-zero
                    # accumulate semantics and Tile's HW sync are unaffected.
                    if c < NCH - 1:
                        for lo, hi in ((0, 512), (512, H)):
                            nc.tensor.matmul(
                                out=M_p[:, lo:hi],
                                lhsT=Ap[:],
                                rhs=BX[:, lo:hi],
                                start=(c == 0),
                                stop=True,
                                skip_group_check=True,
                            )

                    # acc = St^T @ BX (+ Et^T @ M)  [C, H]
                    out_p = ps_out.tile([C, H], f32, name="out_p", tag="out_p")
                    for lo, hi in ((0, 512), (512, H)):
                        nc.tensor.matmul(
                            out=out_p[:, lo:hi],
                            lhsT=St[:],
                            rhs=BX[:, lo:hi],
                            start=True,
                            stop=(c == 0),
                        )
                    if c > 0:
                        for lo, hi in ((0, 512), (512, H)):
                            nc.tensor.matmul(
                                out=out_p[:, lo:hi],
                                lhsT=Et,
                                rhs=M_s[:, lo:hi],
                                start=False,
                                stop=True,
                            )

                    # out = acc * (1/(j+1)) + bx  (bf16 out; host upcasts)
                    if c % IOCH == 0:
                        OUTW = outpool.tile([C, IOCH * H], bf16, name="OUTW", tag="OUTW")
                    out_s = OUTW[:, (c % IOCH) * H:(c % IOCH + 1) * H]
                    nc.vector.scalar_tensor_tensor(
                        out=out_s,
                        in0=out_p[:],
                        scalar=consts_s[:, NCH + c:NCH + c + 1],
                        in1=BX,
                        op0=mult,
                        op1=add,
                    )
                    # stores go on the ACT HWDGE ring so they don't FIFO-block
                    # the bx loads on the SP ring
                    if c % IOCH == IOCH - 1:
                        nc.scalar.dma_start(
                            out=out_d[(g - IOCH + 1) * C:(g + 1) * C, :].rearrange(
                                "(k p) h -> p k h", k=IOCH
                            ),
                            in_=OUTW[:].rearrange("p (k h) -> p k h", k=IOCH),
                        )

    # Adjacent PE matmuls sharing a stationary operand reload it redundantly;
    # mark the second of each such pair as pre-loaded (ldweights=True).
    import concourse.mybir as mybir

    for blk in nc.m.functions[0].blocks:
        last = None
        for inst in blk.instructions:
            if getattr(inst, "engine", None) != mybir.EngineType.PE:
                continue
            if not isinstance(inst, mybir.InstMatmult):
                if isinstance(inst, (mybir.InstLdweights,)):
                    last = None
                continue
            if (
                last is not None
                and not inst.is_transpose
                and not last.is_transpose
                and inst.ins[1].memref == last.ins[1].memref
                and inst.ins[1].offset == last.ins[1].offset
                and inst.ins[1].ap == last.ins[1].ap
            ):
                inst.ldweights = True
            last = inst

    nc.compile()
    _compiled[key] = nc
    return nc


def _pack_idx(x):
    """[128, BPC*NI16] int16: batch b cols [b*NI16:(b+1)*NI16], 16-part wrap,
    replicated 8x down the partitions."""
    cols = []
    for b in range(x.shape[0]):
        base = x[b].astype(np.int16).reshape(NI16, 16).T  # [16, NI16]
        cols.append(np.tile(base, (8, 1)))  # [128, NI16]
    return np.ascontiguousarray(np.concatenate(cols, axis=1))


def _in_maps(bert_x, x, ae, w):
    import ml_dtypes

    bert_x = np.ascontiguousarray(
        np.asarray(bert_x, dtype=np.float32).astype(ml_dtypes.bfloat16)
    )
    x = np.asarray(x)
    ae = np.asarray(ae, dtype=np.float32)
    w = np.asarray(w, dtype=np.float32)
    eaw = np.ascontiguousarray(
        np.concatenate([ae, ae @ w], axis=1).astype(ml_dtypes.bfloat16)
    )
    cmask, consts = _np_consts()
    maps = []
    for k in range(NCORES):
        maps.append(
            {
                "bx": bert_x[k * BPC:(k + 1) * BPC].reshape(ROWS, H),
                "idx": _pack_idx(x[k * BPC:(k + 1) * BPC]),
                "eaw": eaw,
                "cmask": cmask,
                "consts": consts,
            }
        )
    return maps


def _run(bert_x, x, ae, w, trace=False):
    from concourse import bass_utils

    nc = _build()
    maps = _in_maps(bert_x, x, ae, w)
    res = bass_utils.run_bass_kernel_spmd(
        nc, maps, core_ids=list(range(NCORES)), trace=trace
    )
    out = np.concatenate(
        [
            np.asarray(res.results[k]["out"])
            .astype(np.float32)
            .reshape(BPC, L, H)
            for k in range(NCORES)
        ],
        axis=0,
    )
    return out, res


def kernel(bert_x, x, ae, w):
    out, _ = _run(bert_x, x, ae, w, trace=False)
    return out


# revision 12
# speedup vs baseline: 1.4105x; 1.4105x over previous
"""Trainium2 Bass kernel for nn_CausalLayer (bilinear causal mixing layer).

Chunked linear-attention identity (see docstring history). Per-chunk indirect
gathers (proven HW path), loads on the SP HWDGE ring, stores on the ACT ring,
bf16 output upcast on host, bf16 cmask.
"""

import os
import sys

for _p in ("/opt/trn_rl_repo", "/root/.axon_site/_ro/trn_rl_repo"):
    if os.path.isdir(_p) and _p not in sys.path:
        sys.path.insert(0, _p)

import numpy as np

B, L, H = 16, 2048, 768
V, D = 30000, 64
NCORES = 8
BPC = B // NCORES
C = 128
NCH = L // C
ROWS = BPC * L
IOCH = 4

_compiled = {}


def _np_consts():
    import ml_dtypes

    i = np.arange(C, dtype=np.float64)
    cmask = np.zeros((C, NCH * C), np.float32)
    consts = np.zeros((C, 2 * NCH), np.float32)
    for c in range(NCH):
        gi = c * C + i
        cmask[:, c * C:(c + 1) * C] = np.where(
            i[:, None] < i[None, :], (gi + 1.0)[:, None], 0.0
        ).astype(np.float32)
        consts[:, c] = (gi + 1.0).astype(np.float32)
        consts[:, NCH + c] = (1.0 / (gi + 1.0)).astype(np.float32)
    return cmask.astype(ml_dtypes.bfloat16), consts


def _build():
    key = "v2c"
    if key in _compiled:
        return _compiled[key]

    import concourse.bacc as bacc
    import concourse.bass as bass
    import concourse.mybir as mybir
    import concourse.tile as tile
    from concourse.masks import make_identity

    f32 = mybir.dt.float32
    i32 = mybir.dt.int32
    bf16 = mybir.dt.bfloat16

    nc = bacc.Bacc(
        "TRN2",
        target_bir_lowering=False,
        debug=False,
        enable_asserts=False,
        num_devices=NCORES,
    )

    bx_d = nc.dram_tensor("bx", [ROWS, H], bf16, kind="ExternalInput").ap()
    idx_d = nc.dram_tensor("idx", [C, BPC * NCH], i32, kind="ExternalInput").ap()
    eaw_d = nc.dram_tensor("eaw", [V, 2 * D], bf16, kind="ExternalInput").ap()
    cm_d = nc.dram_tensor("cmask", [C, NCH * C], bf16, kind="ExternalInput").ap()
    ct_d = nc.dram_tensor("consts", [C, 2 * NCH], f32, kind="ExternalInput").ap()
    out_d = nc.dram_tensor("out", [ROWS, H], bf16, kind="ExternalOutput").ap()

    mult = mybir.AluOpType.mult
    add = mybir.AluOpType.add

    with tile.TileContext(nc) as tc:
        with (
            tc.tile_pool(name="const", bufs=1) as cpool,
            tc.tile_pool(name="bxp", bufs=3) as bxpool,
            tc.tile_pool(name="outp", bufs=2) as outpool,
            tc.tile_pool(name="sm", bufs=4) as smpool,
            tc.tile_pool(name="eap", bufs=6) as eapool,
            tc.tile_pool(name="mp", bufs=2) as mpool,
            tc.tile_pool(name="ps_ea", bufs=1, space="PSUM") as ps_ea,
            tc.tile_pool(name="ps_s", bufs=1, space="PSUM") as ps_s,
            tc.tile_pool(name="ps_out", bufs=2, space="PSUM") as ps_out,
            tc.tile_pool(name="ps_m", bufs=1, space="PSUM") as ps_m,
        ):
            ident16 = cpool.tile([C, C], bf16)
            make_identity(nc, ident16[:])
            idx_s = cpool.tile([C, BPC * NCH], i32)
            nc.sync.dma_start(out=idx_s[:], in_=idx_d[:, :])
            consts_s = cpool.tile([C, 2 * NCH], f32)
            nc.sync.dma_start(out=consts_s[:], in_=ct_d[:, :])
            # cmask rides the (idle at start) ACT ring
            cmask_s = cpool.tile([C, NCH * C], bf16)
            nc.scalar.dma_start(out=cmask_s[:], in_=cm_d[:, :])

            for b in range(BPC):
                M_p = ps_m.tile([D, H], f32, name=f"M_p_b{b}", tag="M_p")
                for c in range(NCH):
                    g = b * NCH + c

                    if c % IOCH == 0:
                        BXW = bxpool.tile([C, IOCH * H], bf16, name="BXW", tag="BXW")
                        nc.sync.dma_start(
                            out=BXW[:].rearrange("p (k h) -> p k h", k=IOCH),
                            in_=bx_d[g * C:(g + IOCH) * C, :].rearrange(
                                "(k p) h -> p k h", k=IOCH
                            ),
                        )
                    BX = BXW[:, (c % IOCH) * H:(c % IOCH + 1) * H]

                    if c > 0:
                        M_s = mpool.tile([D, H], bf16, name="M_s", tag="M_s")
                        nc.scalar.copy(out=M_s[:], in_=M_p[:])

                    EA = eapool.tile([C, 2 * D], bf16, name="EA", tag="EA")
                    nc.gpsimd.indirect_dma_start(
                        out=EA[:],
                        out_offset=None,
                        in_=eaw_d[:, :],
                        in_offset=bass.IndirectOffsetOnAxis(
                            ap=idx_s[:, g:g + 1], axis=0
                        ),
                    )

                    ea_p = ps_ea.tile([D, 2 * C], bf16, name="ea_p", tag="ea_p")
                    nc.tensor.transpose(
                        out=ea_p[:, 0:C], in_=EA[:, 0:D], identity=ident16[:]
                    )
                    nc.tensor.transpose(
                        out=ea_p[:, C:2 * C], in_=EA[:, D:2 * D], identity=ident16[:]
                    )
                    Et = smpool.tile([D, C], bf16, name="Et", tag="Et")
                    nc.scalar.copy(out=Et[:], in_=ea_p[:, 0:C])
                    At = smpool.tile([D, C], bf16, name="At", tag="At")
                    nc.vector.tensor_scalar_mul(
                        out=At[:], in0=ea_p[:, C:2 * C], scalar1=1.0
                    )

                    # Ap = A * (i+1)  [C, D]
                    Ap = smpool.tile([C, D], bf16, name="Ap", tag="Ap")
                    nc.vector.tensor_scalar_mul(
                        out=Ap[:], in0=EA[:, D:2 * D], scalar1=consts_s[:, c:c + 1]
                    )

                    s_p = ps_s.tile([C, C], f32, name="s_p", tag="s_p")
                    nc.tensor.matmul(
                        out=s_p[:], lhsT=At[:], rhs=Et[:], start=True, stop=True,
                    )
                    St = smpool.tile([C, C], bf16, name="St", tag="St")
                    nc.vector.tensor_tensor(
                        out=St[:],
                        in0=s_p[:],
                        in1=cmask_s[:, c * C:(c + 1) * C],
                        op=mult,
                    )

                    if c < NCH - 1:
                        for lo, hi in ((0, 512), (512, H)):
                            nc.tensor.matmul(
                                out=M_p[:, lo:hi],
                                lhsT=Ap[:],
                                rhs=BX[:, lo:hi],
                                start=(c == 0),
                                stop=True,
                                skip_group_check=True,
                            )

                    out_p = ps_out.tile([C, H], f32, name="out_p", tag="out_p")
                    for lo, hi in ((0, 512), (512, H)):
                        nc.tensor.matmul(
                            out=out_p[:, lo:hi],
                            lhsT=St[:],
                            rhs=BX[:, lo:hi],
                            start=True,
                            stop=(c == 0),
                        )
                    if c > 0:
                        for lo, hi in ((0, 512), (512, H)):
                            nc.tensor.matmul(
                                out=out_p[:, lo:hi],
                                lhsT=Et[:],
                                rhs=M_s[:, lo:hi],
                                start=False,
                                stop=True,
                            )

                    if c % IOCH == 0:
                        OUTW = outpool.tile([C, IOCH * H], bf16, name="OUTW", tag="OUTW")
                    out_s = OUTW[:, (c % IOCH) * H:(c % IOCH + 1) * H]
                    nc.vector.scalar_tensor_tensor(
                        out=out_s,
                        in0=out_p[:],
                        scalar=consts_s[:, NCH + c:NCH + c + 1],
                        in1=BX,
                        op0=mult,
                        op1=add,
                    )
                    if c % IOCH == IOCH - 1:
                        nc.scalar.dma_start(
                            out=out_d[(g - IOCH + 1) * C:(g + 1) * C, :].rearrange(
                                "(k p) h -> p k h", k=IOCH
                            ),
                            in_=OUTW[:].rearrange("p (k h) -> p k h", k=IOCH),
                        )

    import concourse.mybir as mybir

    for blk in nc.m.functions[0].blocks:
        last = None
        for inst in blk.instructions:
            if getattr(inst, "engine", None) != mybir.EngineType.PE:
                continue
            if not isinstance(inst, mybir.InstMatmult):
                if isinstance(inst, (mybir.InstLdweights,)):
                    last = None
                continue
            if (
                last is not None
                and not inst.is_transpose
                and not last.is_transpose
                and inst.ins[1].memref == last.ins[1].memref
                and inst.ins[1].offset == last.ins[1].offset
                and inst.ins[1].ap == last.ins[1].ap
            ):
                inst.ldweights = True
            last = inst

    nc.compile()
    _compiled[key] = nc
    return nc


def _in_maps(bert_x, x, ae, w):
    import ml_dtypes

    bert_x = np.ascontiguousarray(
        np.asarray(bert_x, dtype=np.float32).astype(ml_dtypes.bfloat16)
    )
    x = np.asarray(x)
    ae = np.asarray(ae, dtype=np.float32)
    w = np.asarray(w, dtype=np.float32)
    eaw = np.ascontiguousarray(
        np.concatenate([ae, ae @ w], axis=1).astype(ml_dtypes.bfloat16)
    )
    cmask, consts = _np_consts()
    xr = x.reshape(B, NCH, C).transpose(0, 2, 1).astype(np.int32)
    maps = []
    for k in range(NCORES):
        maps.append(
            {
                "bx": bert_x[k * BPC:(k + 1) * BPC].reshape(ROWS, H),
                "idx": np.ascontiguousarray(
                    np.concatenate([xr[k * BPC + b] for b in range(BPC)], axis=1)
                ),
                "eaw": eaw,
                "cmask": cmask,
                "consts": consts,
            }
        )
    return maps


def _run(bert_x, x, ae, w, trace=False):
    from concourse import bass_utils

    nc = _build()
    maps = _in_maps(bert_x, x, ae, w)
    res = bass_utils.run_bass_kernel_spmd(
        nc, maps, core_ids=list(range(NCORES)), trace=trace
    )
    out = np.concatenate(
        [
            np.asarray(res.results[k]["out"])
            .astype(np.float32)
            .reshape(BPC, L, H)
            for k in range(NCORES)
        ],
        axis=0,
    )
    return out, res


def kernel(bert_x, x, ae, w):
    out, _ = _run(bert_x, x, ae, w, trace=False)
    return out
